# revision 22
# baseline (speedup 1.0000x reference)
"""ComplexLayerNorm Trainium2 kernel (8 NeuronCores, SPMD, C-sharded).

Math (see reference): per-feature 2x2 covariance whitening of (re, im) over
all B*C samples (centered with the batch-only mean mu_b), after subtracting
the complex mean over F, plus complex affine.

Redesign vs the fp32 baseline (362 us -> target ~110 us), driven by the tile
cost model:
  * All PE work in float32r: non-transpose matmuls with out-free >= 256 run
    at 1 cycle/row (vs 4 for fp32); PE-transpose instructions at 1.5.
  * f-slice streaming: x is loaded in 256-feature column slices, so the
    (f,2,2) covariance partials complete per-slice; the cross-core reduction
    is split into two AllGathers (no 1.875x AllReduce tax) that overlap the
    second half of the input stream and the mean matmuls.
  * T[c,f] (batch sums) via tiny fp32r matmuls (rhs = 16-col selector) with
    uninterrupted PSUM accumulation groups per f-chunk.
  * Complex mean over F via ones-matmuls accumulated in PSUM across all 16
    f-chunks (negated ones -> M3 rows directly).
  * Apply phase unchanged in spirit (diagonal-W matmuls emitting interleaved
    (f,2) output) but fp32r, with PSUM->SBUF staging copies spread over
    DVE/ACT and 1MB stores alternating between the SP/ACT DMA rings.

Sharding: C (=128) split 16-per-core; host slices inputs and concatenates
the 8 C-shards of the (B,C,F,2) output.
"""

import numpy as np

import bass_rust
import concourse.bass as bass
import concourse.mybir as mybir
from concourse import tile
from concourse.bass_utils import run_bass_kernel_spmd


def split_multi_waits(nc):
    """The walrus build in this container allows only ONE sync-wait command
    per instruction; Tile emits several.  Split extras into preceding
    single-wait NoOps on the same engine (sequential waits == AND)."""
    cnt = 0
    for bb in nc.main_func.blocks:
        il = bb.instructions
        newlist = []
        changed = False
        for inst in list(il):
            si = inst.sync_info
            waits = list(si.on_wait) if si else []
            if len(waits) > 1:
                changed = True
                for w in waits[:-1]:
                    cnt += 1
                    nop = bass_rust.InstNoOp(name=f"I-wsplit-{cnt}")
                    nop.engine = inst.engine
                    nop.sync_info = mybir.SyncInfo(on_wait=[w], on_update=[])
                    newlist.append(nop)
                inst.sync_info = mybir.SyncInfo(
                    on_wait=[waits[-1]], on_update=list(si.on_update))
            newlist.append(inst)
        if changed:
            il[:] = newlist
    return cnt


FP = mybir.dt.float32
FPR = mybir.dt.float32r
BF = mybir.dt.bfloat16
AF = mybir.ActivationFunctionType
OP = mybir.AluOpType

B, C, F = 64, 128, 2048
NCORES = 8
CSH = C // NCORES           # 16 channels per core
BC = B * CSH                # 1024 sample rows per core
NFT = F // 128              # 16 f-chunks
NSP = 8                     # slice-pairs of 256 features (2 chunks) each
EPS = 1e-4
NM1 = float(B * C - 1)      # 8191


def build_bass():
    nc = bass.Bass()

    x_r = nc.dram_tensor("x_r", [BC, F], FPR, kind="ExternalInput")
    x_i = nc.dram_tensor("x_i", [BC, F], FPR, kind="ExternalInput")
    # gamma pre-tiled on host to (128, NFT): tile[p, t] = gamma[128*t + p]
    g_r = nc.dram_tensor("g_r", [128, NFT], FP, kind="ExternalInput")
    g_i = nc.dram_tensor("g_i", [128, NFT], FP, kind="ExternalInput")
    # beta interleaved on host: (1, 4096) = [b_r[0], b_i[0], b_r[1], ...]
    beta_ilv = nc.dram_tensor("beta_ilv", [1, 2 * F], BF,
                              kind="ExternalInput")
    ident = nc.dram_tensor("ident", [128, 128], FPR, kind="ExternalInput")
    seldr = nc.dram_tensor("seldr", [128, CSH], FPR, kind="ExternalInput")
    onesF = nc.dram_tensor("onesF", [128, 1], FPR, kind="ExternalInput")
    ones_bc = nc.dram_tensor("ones_bc", [1, BC], BF, kind="ExternalInput")

    out = nc.dram_tensor("out", [BC, 2 * F], FP, kind="ExternalOutput")

    from contextlib import ExitStack

    with tile.TileContext(nc) as tc:
        with (
            tc.tile_pool(name="big", bufs=1) as big,
            tc.tile_pool(name="small", bufs=1) as small,
            tc.tile_pool(name="wpool", bufs=2) as wpool,
            tc.tile_pool(name="dram", bufs=1, space="DRAM") as dram,
        ):
            # ---- constants to SBUF
            ident_t = small.tile([128, 128], FPR, tag="ident")
            nc.scalar.dma_start(ident_t[:], ident[:])
            sel_t = small.tile([128, CSH], FPR, tag="sel")
            nc.scalar.dma_start(sel_t[:], seldr[:])
            onesF_t = small.tile([128, 1], FPR, tag="onesF")
            nc.scalar.dma_start(onesF_t[:], onesF[:])
            g_r_t = small.tile([128, NFT], FP, tag="g_r")
            nc.scalar.dma_start(g_r_t[:], g_r[:])
            g_i_t = small.tile([128, NFT], FP, tag="g_i")
            nc.scalar.dma_start(g_i_t[:], g_i[:])

            # A3C rhs for the K=3 correction matmul: (3, 4096)
            # row0[2f+c] = (a_rr, a_ir)[c][f]; row1: (a_ri, a_ii); row2: beta
            A3C = small.tile([3, 2 * F], BF, tag="A3C")
            # M3 = [-mean_r; -mean_i; ones]  (3, 1024); ones row is static.
            M3 = small.tile([3, BC], BF, tag="M3")

            # ---- persistent: x transposed, xT[p, 1024*t + j] = x[j, 128*t+p]
            xT_r = big.tile([128, NFT * BC], FPR, tag="xT_r")
            xT_i = big.tile([128, NFT * BC], FPR, tag="xT_i")
            xT = {"r": xT_r, "i": xT_i}

            # second-moment accumulators, one column per f-chunk
            S_rr = small.tile([128, NFT], FP, tag="S_rr")
            S_ri = small.tile([128, NFT], FP, tag="S_ri")
            S_ii = small.tile([128, NFT], FP, tag="S_ii")
            corr_rr = small.tile([128, NFT], FP, tag="corr_rr")
            corr_ri = small.tile([128, NFT], FP, tag="corr_ri")
            corr_ii = small.tile([128, NFT], FP, tag="corr_ii")

            _stk = ExitStack()
            natp = {
                "r": _stk.enter_context(tc.tile_pool(name="natr", bufs=2)),
                "i": _stk.enter_context(tc.tile_pool(name="nati", bufs=2)),
            }
            scrp = _stk.enter_context(tc.tile_pool(name="scr", bufs=2))
            psT = _stk.enter_context(
                tc.tile_pool(name="psT", bufs=2, space="PSUM"))
            psX = _stk.enter_context(
                tc.tile_pool(name="psX", bufs=2, space="PSUM"))
            mean_stk = ExitStack()
            psM = mean_stk.enter_context(
                tc.tile_pool(name="psM", bufs=1, space="PSUM"))
            mean_ps = {"r": psM.tile([1, BC], FP, tag="mean_r",
                                     name="mean_ps_r"),
                       "i": psM.tile([1, BC], FP, tag="mean_i",
                                     name="mean_ps_i")}

            xdr = {"r": x_r, "i": x_i}
            # copy-engine rotation for the PSUM->SBUF transpose copies
            copy_rot = ["v", "v", "v", "a", "v", "v", "v", "a"]
            cov_h = [None, None]
            ag_wait = []

            # ======== Phase A: stream f-slices; transpose, T, S, mean ======
            for s in range(NSP):
                t0 = 2 * s
                nat = {}
                for ch in ("r", "i"):
                    n = natp[ch].tile([128, 8 * 256], FPR, tag="nat")
                    src = xdr[ch][:, 256 * s:256 * (s + 1)].rearrange(
                        "(a p) f -> p a f", p=128)
                    dst = n[:].rearrange("p (a f) -> p a f", f=256)
                    nc.sync.dma_start(dst, src)
                    nat[ch] = n

                # --- T[c,f] via fp32r matmuls, one uninterrupted group per
                # (channel, chunk); the T-quadratic corr terms read the PSUM
                # tiles directly (no SBUF copy of T needed).
                for tt in range(2):
                    t = t0 + tt
                    pT = {}
                    for ch in ("r", "i"):
                        pTc = psT.tile([128, CSH], FP, tag="pT",
                                       name=f"pT_{ch}_{t}")
                        for b in range(8):
                            lhsT = nat[ch][:, 256 * b + 128 * tt:
                                           256 * b + 128 * (tt + 1)]
                            nc.tensor.matmul(pTc[:], lhsT, sel_t[:],
                                             start=(b == 0), stop=(b == 7))
                        pT[ch] = pTc
                    ts1 = scrp.tile([128, CSH], FP, tag="tsq")
                    nc.scalar.activation(ts1[:], pT["r"][:], AF.Square,
                                         accum_out=corr_rr[:, t:t + 1])
                    ts2 = scrp.tile([128, CSH], FP, tag="tsq")
                    nc.scalar.activation(ts2[:], pT["i"][:], AF.Square,
                                         accum_out=corr_ii[:, t:t + 1])
                    tr_sb = scrp.tile([128, CSH], FP, tag="tsq")
                    nc.vector.tensor_copy(tr_sb[:], pT["r"][:])
                    ts3 = scrp.tile([128, CSH], FP, tag="tsq")
                    nc.vector.scalar_tensor_tensor(
                        out=ts3[:], in0=tr_sb[:], scalar=1.0,
                        in1=pT["i"][:], op0=OP.mult, op1=OP.mult,
                        accum_out=corr_ri[:, t:t + 1],
                    )

                # --- transposes: 4 per psum tile (2 b x 2 chunks), then one
                # strided 512-col copy into xT.
                ci = 0
                for ch in ("r", "i"):
                    for bp in range(4):
                        px = psX.tile([128, 512], FPR, tag="px")
                        for bb in range(2):
                            for tt in range(2):
                                b = 2 * bp + bb
                                nc.tensor.transpose(
                                    px[:, 256 * tt + 128 * bb:
                                       256 * tt + 128 * bb + 128],
                                    nat[ch][:, 256 * b + 128 * tt:
                                            256 * b + 128 * (tt + 1)],
                                    ident_t[:],
                                )
                        dst = xT[ch][:].rearrange(
                            "p (t j) -> p t j", j=BC
                        )[:, t0:t0 + 2, 256 * bp:256 * (bp + 1)]
                        srcv = px[:].rearrange("p (t q) -> p t q", q=256)
                        eng = copy_rot[ci]
                        ci += 1
                        if eng == "v":
                            nc.vector.tensor_copy(dst, srcv)
                        elif eng == "a":
                            nc.scalar.copy(dst, srcv)
                        else:
                            nc.gpsimd.tensor_copy(dst, srcv)

                # --- mean over F: negated ones-matmuls accumulated in PSUM
                # across all chunks (groups span phase A; own psum banks).
                for ch in ("r", "i"):
                    for tt in range(2):
                        t = t0 + tt
                        for h in range(2):
                            nc.tensor.matmul(
                                mean_ps[ch][:, 512 * h:512 * (h + 1)],
                                onesF_t[:],
                                xT[ch][:, BC * t + 512 * h:
                                       BC * t + 512 * (h + 1)],
                                start=(t == 0), stop=(t == NFT - 1),
                            )

                # --- second moments per chunk (full-bc accumulation in one
                # instruction each) + T-quadratic correction.
                for tt in range(2):
                    t = t0 + tt
                    sl = slice(BC * t, BC * (t + 1))
                    sc1 = scrp.tile([128, BC], FP, tag="sq")
                    nc.scalar.activation(sc1[:], xT_r[:, sl], AF.Square,
                                         accum_out=S_rr[:, t:t + 1])
                    sc2 = scrp.tile([128, BC], FP, tag="sq")
                    nc.scalar.activation(sc2[:], xT_i[:, sl], AF.Square,
                                         accum_out=S_ii[:, t:t + 1])
                    sc3 = scrp.tile([128, BC], FP, tag="sq")
                    nc.gpsimd.tensor_tensor(out=sc3[:], in0=xT_r[:, sl],
                                            in1=xT_i[:, sl], op=OP.mult)
                    nc.vector.tensor_reduce(
                        S_ri[:, t:t + 1], sc3[:], mybir.AxisListType.X,
                        OP.add)

                # --- at half boundaries: pack raw partial cov, AllGather it
                # (bounce DMAs on the gpsimd SWDGE ring so they never block
                # the SP input-load stream).
                if s in (3, 7):
                    h = s // 4
                    hs = slice(8 * h, 8 * (h + 1))
                    partial = small.tile([128, 24], FP, tag=f"partial{h}",
                                         name=f"partial{h}")
                    for m, (S, corr) in enumerate(
                        ((S_rr, corr_rr), (S_ri, corr_ri), (S_ii, corr_ii))
                    ):
                        nc.vector.scalar_tensor_tensor(
                            out=partial[:, 8 * m:8 * (m + 1)],
                            in0=corr[:, hs], scalar=-1.0 / B, in1=S[:, hs],
                            op0=OP.mult, op1=OP.add,
                        )
                    ag_in = dram.tile([128, 24], FP, tag=f"ag_in{h}",
                                      name=f"ag_in{h}")
                    ag_out = dram.tile([8, 128 * 24], FP, tag=f"ag_out{h}",
                                       name=f"ag_out{h}")
                    nc.gpsimd.dma_start(ag_in[:], partial[:])
                    nc.gpsimd.collective_compute(
                        "AllGather", OP.bypass,
                        replica_groups=[list(range(NCORES))],
                        ins=[ag_in[:]],
                        outs=[ag_out[:]],
                    )
                    ag_wait.append(ag_out)

            # ======== M3: copy negated means out of PSUM, DMA into rows ====
            for r, ch in ((0, "r"), (1, "i")):
                row = small.tile([1, BC], BF, tag="rowtmp", name=f"row{ch}")
                nc.vector.tensor_copy(row[:], mean_ps[ch][:])
                nc.sync.dma_start(M3[r:r + 1, :], row[:])
            mean_stk.close()
            _stk.close()

            # beta / ones rows (bf16, late on the ACT ring: they only gate
            # the K3 matmuls of phase D).
            nc.scalar.dma_start(A3C[2:3, :], beta_ilv[:])
            nc.scalar.dma_start(M3[2:3, :], ones_bc[:])

            # ======== Phases C+D interleaved per half, so half 0 applies
            # while half 1's AllGather is still in flight.
            a_rr = small.tile([128, NFT], FP, tag="a_rr")
            a_ri = small.tile([128, NFT], FP, tag="a_ri")
            a_ir = small.tile([128, NFT], FP, tag="a_ir")
            a_ii = small.tile([128, NFT], FP, tag="a_ii")

            _stk3 = ExitStack()
            pso = _stk3.enter_context(
                tc.tile_pool(name="pso", bufs=5, space="PSUM"))
            stgp = _stk3.enter_context(tc.tile_pool(name="stg", bufs=3))
            copy_flip = 0

            for h in range(2):
                hs = slice(8 * h, 8 * (h + 1))
                # gather result -> SBUF (gpsimd ring; Pool is idle by now)
                # and tree-reduce the 8 replica partials.
                gat = small.tile([128, 8 * 24], FP, tag=f"gat{h}",
                                 name=f"gat{h}")
                nc.gpsimd.dma_start(
                    gat[:].rearrange("p (r j) -> p r j", j=24),
                    ag_wait[h][:].rearrange("r (p j) -> p r j", p=128),
                )
                nc.vector.tensor_tensor(out=gat[:, 0:96], in0=gat[:, 0:96],
                                        in1=gat[:, 96:192], op=OP.add)
                nc.vector.tensor_tensor(out=gat[:, 0:48], in0=gat[:, 0:48],
                                        in1=gat[:, 48:96], op=OP.add)
                cov = small.tile([128, 24], FP, tag=f"cov{h}",
                                 name=f"cov{h}")
                nc.vector.tensor_tensor(out=cov[:], in0=gat[:, 0:24],
                                        in1=gat[:, 24:48], op=OP.add)

                def stile(tag):
                    return small.tile([128, 8], FP, tag=tag,
                                      name=f"{tag}_{h}")

                arr, bri, cii = stile("arr"), stile("bri"), stile("cii")
                nc.vector.tensor_scalar(out=arr[:], in0=cov[:, 0:8],
                                        scalar1=1.0 / NM1, scalar2=EPS,
                                        op0=OP.mult, op1=OP.add)
                nc.vector.tensor_scalar(out=bri[:], in0=cov[:, 8:16],
                                        scalar1=1.0 / NM1, scalar2=None,
                                        op0=OP.mult)
                nc.vector.tensor_scalar(out=cii[:], in0=cov[:, 16:24],
                                        scalar1=1.0 / NM1, scalar2=EPS,
                                        op0=OP.mult, op1=OP.add)

                det, tmp = stile("det"), stile("tmp")
                nc.vector.tensor_tensor(out=det[:], in0=arr[:], in1=cii[:],
                                        op=OP.mult)
                nc.vector.tensor_tensor(out=tmp[:], in0=bri[:], in1=bri[:],
                                        op=OP.mult)
                nc.vector.tensor_tensor(out=det[:], in0=det[:], in1=tmp[:],
                                        op=OP.subtract)
                s_t = stile("s_t")
                nc.scalar.activation(s_t[:], det[:], AF.Sqrt)
                tsum = stile("tsum")
                nc.vector.tensor_tensor(out=tsum[:], in0=arr[:], in1=cii[:],
                                        op=OP.add)
                nc.vector.scalar_tensor_tensor(
                    out=tsum[:], in0=s_t[:], scalar=2.0, in1=tsum[:],
                    op0=OP.mult, op1=OP.add)
                tval = stile("tval")
                nc.scalar.activation(tval[:], tsum[:], AF.Sqrt)
                den, rden = stile("den"), stile("rden")
                nc.vector.tensor_tensor(out=den[:], in0=s_t[:], in1=tval[:],
                                        op=OP.mult)
                nc.vector.reciprocal(rden[:], den[:])

                w_rr, w_ii, wri_n = stile("w_rr"), stile("w_ii"), stile("wri")
                nc.vector.tensor_tensor(out=w_rr[:], in0=cii[:], in1=s_t[:],
                                        op=OP.add)
                nc.vector.tensor_tensor(out=w_rr[:], in0=w_rr[:], in1=rden[:],
                                        op=OP.mult)
                nc.vector.tensor_tensor(out=w_ii[:], in0=arr[:], in1=s_t[:],
                                        op=OP.add)
                nc.vector.tensor_tensor(out=w_ii[:], in0=w_ii[:], in1=rden[:],
                                        op=OP.mult)
                nc.vector.scalar_tensor_tensor(
                    out=wri_n[:], in0=bri[:], scalar=-1.0, in1=rden[:],
                    op0=OP.mult, op1=OP.mult)

                u, v = stile("u"), stile("v")
                for a_t, gx, wx, gy, wy, opc in (
                    (a_rr, g_r_t, w_rr, g_i_t, wri_n, OP.subtract),
                    (a_ri, g_r_t, wri_n, g_i_t, w_ii, OP.subtract),
                    (a_ir, g_i_t, w_rr, g_r_t, wri_n, OP.add),
                    (a_ii, g_i_t, wri_n, g_r_t, w_ii, OP.add),
                ):
                    nc.vector.tensor_tensor(out=u[:], in0=gx[:, hs],
                                            in1=wx[:], op=OP.mult)
                    nc.vector.tensor_tensor(out=v[:], in0=gy[:, hs],
                                            in1=wy[:], op=OP.mult)
                    nc.vector.tensor_tensor(out=a_t[:, hs], in0=u[:],
                                            in1=v[:], op=opc)

                # A3C rows for this half: bf16 staging copy, DRAM bounce,
                # strided re-read into the interleaved row layout.
                for row, (ev, od) in enumerate(((a_rr, a_ir), (a_ri, a_ii))):
                    for cpar, srct in ((0, ev), (1, od)):
                        abf = small.tile([128, 8], BF,
                                         tag=f"abf{h}{row}{cpar}",
                                         name=f"abf{h}{row}{cpar}")
                        nc.vector.tensor_copy(abf[:], srct[:, hs])
                        db = dram.tile([128, 8], BF, tag=f"db{h}{row}{cpar}",
                                       name=f"db{h}{row}{cpar}")
                        nc.sync.dma_start(db[:], abf[:])
                        src = db[:].rearrange("p t -> (p t)").rearrange(
                            "(p t) -> t p", p=128, t=8)
                        dst = A3C[row:row + 1,
                                  2048 * h + cpar:2048 * (h + 1):2].rearrange(
                            "z (t p) -> z t p", t=8, p=128)
                        nc.sync.dma_start(dst, src)

                # ---- Phase D for this half: diag-W apply, stage, store
                for t2 in range(4 * h, 4 * h + 4):
                    ta, tb = 2 * t2, 2 * t2 + 1
                    Ws = []
                    for t in (ta, tb):
                        W_r = wpool.tile([128, 256], FPR, tag="W_r",
                                         name=f"W_r_{t}")
                        W_i = wpool.tile([128, 256], FPR, tag="W_i",
                                         name=f"W_i_{t}")
                        for W, (ev, od) in ((W_r, (a_rr, a_ir)),
                                            (W_i, (a_ri, a_ii))):
                            Wv = W[:].rearrange("p (g c) -> p g c", c=2)
                            nc.vector.tensor_scalar(
                                out=Wv[:, :, 0], in0=ident_t[:],
                                scalar1=ev[:, t:t + 1], scalar2=None,
                                op0=OP.mult,
                            )
                            nc.vector.tensor_scalar(
                                out=Wv[:, :, 1], in0=ident_t[:],
                                scalar1=od[:, t:t + 1], scalar2=None,
                                op0=OP.mult,
                            )
                        Ws.append((W_r, W_i))
                    for bp in range(4):
                        stg = stgp.tile([128, 1024], FP, tag="stg")
                        for bb in range(2):
                            b = 2 * bp + bb
                            po = pso.tile([128, 512], FP, tag="po")
                            nc.tensor.matmul(
                                po[:],
                                M3[:, 128 * b:128 * (b + 1)],
                                A3C[:, 512 * t2:512 * (t2 + 1)],
                                start=True, stop=False,
                            )
                            for j, t in enumerate((ta, tb)):
                                W_r, W_i = Ws[j]
                                sl = slice(BC * t + 128 * b,
                                           BC * t + 128 * (b + 1))
                                nc.tensor.matmul(
                                    po[:, 256 * j:256 * (j + 1)],
                                    xT_r[:, sl], W_r[:],
                                    start=False, stop=False,
                                )
                                nc.tensor.matmul(
                                    po[:, 256 * j:256 * (j + 1)],
                                    xT_i[:, sl], W_i[:],
                                    start=False, stop=(j == 1),
                                )
                            if copy_flip % 2 == 0:
                                nc.vector.tensor_copy(
                                    stg[:, 512 * bb:512 * (bb + 1)], po[:])
                            else:
                                nc.scalar.copy(
                                    stg[:, 512 * bb:512 * (bb + 1)], po[:])
                            copy_flip += 1
                        dst = out.rearrange("(a p) f -> p a f", p=128)[
                            :, 2 * bp:2 * (bp + 1), 512 * t2:512 * (t2 + 1)
                        ]
                        src = stg[:].rearrange("p (a q) -> p a q", q=512)
                        # SP carries the late (half-1) stores only: it is
                        # blocked on gat1 until AG1 completes.
                        if h == 0 or (4 * t2 + bp) % 2 == 0:
                            nc.scalar.dma_start(dst, src)
                        else:
                            nc.sync.dma_start(dst, src)
            _stk3.close()

    split_multi_waits(nc)
    return nc


_CACHE = {}


def _get_nc():
    if "nc" not in _CACHE:
        _CACHE["nc"] = build_bass()
    return _CACHE["nc"]


def _constants():
    if "consts" not in _CACHE:
        sel = np.zeros((128, CSH), dtype=np.float32)
        for p in range(128):
            sel[p, p % CSH] = 1.0
        _CACHE["consts"] = {
            "ident": np.eye(128, dtype=np.float32),
            "seldr": np.ascontiguousarray(sel),
            "onesF": np.full((128, 1), -1.0 / F, dtype=np.float32),
            "ones_bc": np.ones((1, BC), dtype=np.float32),
        }
    return _CACHE["consts"]


def kernel(x_real, x_imag, gamma_r, gamma_i, beta_r, beta_i):
    x_real = np.ascontiguousarray(x_real, dtype=np.float32)
    x_imag = np.ascontiguousarray(x_imag, dtype=np.float32)
    gamma_r = np.asarray(gamma_r, dtype=np.float32)
    gamma_i = np.asarray(gamma_i, dtype=np.float32)
    beta_r = np.asarray(beta_r, dtype=np.float32)
    beta_i = np.asarray(beta_i, dtype=np.float32)

    nc = _get_nc()
    consts = _constants()
    g_r_t = np.ascontiguousarray(gamma_r.reshape(NFT, 128).T)
    g_i_t = np.ascontiguousarray(gamma_i.reshape(NFT, 128).T)
    import ml_dtypes
    beta_ilv = np.ascontiguousarray(
        np.stack([beta_r, beta_i], axis=-1).reshape(1, 2 * F)
    ).astype(ml_dtypes.bfloat16)

    in_maps = []
    for k in range(NCORES):
        cs = slice(CSH * k, CSH * (k + 1))
        in_maps.append({
            "x_r": np.ascontiguousarray(
                x_real[:, cs, :].reshape(BC, F)),
            "x_i": np.ascontiguousarray(
                x_imag[:, cs, :].reshape(BC, F)),
            "g_r": g_r_t, "g_i": g_i_t, "beta_ilv": beta_ilv,
            **consts,
        })

    res = run_bass_kernel_spmd(nc, in_maps, list(range(NCORES)))

    full = np.empty((B, C, F, 2), dtype=np.float32)
    for k in range(NCORES):
        full[:, CSH * k:CSH * (k + 1)] = (
            res.results[k]["out"].reshape(B, CSH, F, 2)
        )
    return full


# revision 23
# speedup vs baseline: 1.1292x; 1.1292x over previous
"""ComplexLayerNorm Trainium2 kernel (8 NeuronCores, SPMD, C-sharded).

Math (see reference): per-feature 2x2 covariance whitening of (re, im) over
all B*C samples (centered with the batch-only mean mu_b), after subtracting
the complex mean over F, plus complex affine.

Redesign vs the fp32 baseline (362 us -> target ~110 us), driven by the tile
cost model:
  * All PE work in float32r: non-transpose matmuls with out-free >= 256 run
    at 1 cycle/row (vs 4 for fp32); PE-transpose instructions at 1.5.
  * f-slice streaming: x is loaded in 256-feature column slices, so the
    (f,2,2) covariance partials complete per-slice; the cross-core reduction
    is split into two AllGathers (no 1.875x AllReduce tax) that overlap the
    second half of the input stream and the mean matmuls.
  * T[c,f] (batch sums) via tiny fp32r matmuls (rhs = 16-col selector) with
    uninterrupted PSUM accumulation groups per f-chunk.
  * Complex mean over F via ones-matmuls accumulated in PSUM across all 16
    f-chunks (negated ones -> M3 rows directly).
  * Apply phase unchanged in spirit (diagonal-W matmuls emitting interleaved
    (f,2) output) but fp32r, with PSUM->SBUF staging copies spread over
    DVE/ACT and 1MB stores alternating between the SP/ACT DMA rings.

Sharding: C (=128) split 16-per-core; host slices inputs and concatenates
the 8 C-shards of the (B,C,F,2) output.
"""

import numpy as np

import bass_rust
import concourse.bass as bass
import concourse.mybir as mybir
from concourse import tile
from concourse.bass_utils import run_bass_kernel_spmd


def split_multi_waits(nc):
    """The walrus build in this container allows only ONE sync-wait command
    per instruction; Tile emits several.  Split extras into preceding
    single-wait NoOps on the same engine (sequential waits == AND)."""
    cnt = 0
    for bb in nc.main_func.blocks:
        il = bb.instructions
        newlist = []
        changed = False
        for inst in list(il):
            si = inst.sync_info
            waits = list(si.on_wait) if si else []
            if len(waits) > 1:
                changed = True
                for w in waits[:-1]:
                    cnt += 1
                    nop = bass_rust.InstNoOp(name=f"I-wsplit-{cnt}")
                    nop.engine = inst.engine
                    nop.sync_info = mybir.SyncInfo(on_wait=[w], on_update=[])
                    newlist.append(nop)
                inst.sync_info = mybir.SyncInfo(
                    on_wait=[waits[-1]], on_update=list(si.on_update))
            newlist.append(inst)
        if changed:
            il[:] = newlist
    return cnt


FP = mybir.dt.float32
FPR = mybir.dt.float32r
BF = mybir.dt.bfloat16
AF = mybir.ActivationFunctionType
OP = mybir.AluOpType

B, C, F = 64, 128, 2048
NCORES = 8
CSH = C // NCORES           # 16 channels per core
BC = B * CSH                # 1024 sample rows per core
NFT = F // 128              # 16 f-chunks
NSP = 8                     # slice-pairs of 256 features (2 chunks) each
EPS = 1e-4
NM1 = float(B * C - 1)      # 8191


def build_bass():
    nc = bass.Bass()

    x_r = nc.dram_tensor("x_r", [BC, F], FPR, kind="ExternalInput")
    x_i = nc.dram_tensor("x_i", [BC, F], FPR, kind="ExternalInput")
    # gamma pre-tiled on host to (128, NFT): tile[p, t] = gamma[128*t + p]
    g_r = nc.dram_tensor("g_r", [128, NFT], FP, kind="ExternalInput")
    g_i = nc.dram_tensor("g_i", [128, NFT], FP, kind="ExternalInput")
    # beta interleaved on host: (1, 4096) = [b_r[0], b_i[0], b_r[1], ...]
    beta_ilv = nc.dram_tensor("beta_ilv", [1, 2 * F], BF,
                              kind="ExternalInput")
    ident = nc.dram_tensor("ident", [128, 128], FPR, kind="ExternalInput")
    seldr = nc.dram_tensor("seldr", [128, CSH], FPR, kind="ExternalInput")
    onesF = nc.dram_tensor("onesF", [128, 1], FPR, kind="ExternalInput")
    ones_bc = nc.dram_tensor("ones_bc", [1, BC], BF, kind="ExternalInput")

    out = nc.dram_tensor("out", [BC, 2 * F], FP, kind="ExternalOutput")

    from contextlib import ExitStack

    with tile.TileContext(nc) as tc:
        with (
            tc.tile_pool(name="big", bufs=1) as big,
            tc.tile_pool(name="small", bufs=1) as small,
            tc.tile_pool(name="wpool", bufs=2) as wpool,
            tc.tile_pool(name="dram", bufs=1, space="DRAM") as dram,
        ):
            # ---- constants to SBUF
            ident_t = small.tile([128, 128], FPR, tag="ident")
            nc.scalar.dma_start(ident_t[:], ident[:])
            sel_t = small.tile([128, CSH], FPR, tag="sel")
            nc.scalar.dma_start(sel_t[:], seldr[:])
            onesF_t = small.tile([128, 1], FPR, tag="onesF")
            nc.scalar.dma_start(onesF_t[:], onesF[:])
            g_r_t = small.tile([128, NFT], FP, tag="g_r")
            nc.scalar.dma_start(g_r_t[:], g_r[:])
            g_i_t = small.tile([128, NFT], FP, tag="g_i")
            nc.scalar.dma_start(g_i_t[:], g_i[:])

            # A3C rhs for the K=3 correction matmul: (3, 4096)
            # row0[2f+c] = (a_rr, a_ir)[c][f]; row1: (a_ri, a_ii); row2: beta
            A3C = small.tile([3, 2 * F], BF, tag="A3C")
            # M3 = [-mean_r; -mean_i; ones]  (3, 1024); ones row is static.
            M3 = small.tile([3, BC], BF, tag="M3")

            # ---- persistent: x transposed, xT[p, 1024*t + j] = x[j, 128*t+p]
            xT_r = big.tile([128, NFT * BC], FPR, tag="xT_r")
            xT_i = big.tile([128, NFT * BC], FPR, tag="xT_i")
            xT = {"r": xT_r, "i": xT_i}

            # second-moment accumulators, one column per f-chunk
            S_rr = small.tile([128, NFT], FP, tag="S_rr")
            S_ri = small.tile([128, NFT], FP, tag="S_ri")
            S_ii = small.tile([128, NFT], FP, tag="S_ii")
            corr_rr = small.tile([128, NFT], FP, tag="corr_rr")
            corr_ri = small.tile([128, NFT], FP, tag="corr_ri")
            corr_ii = small.tile([128, NFT], FP, tag="corr_ii")

            _stk = ExitStack()
            natp = {
                "r": _stk.enter_context(tc.tile_pool(name="natr", bufs=2)),
                "i": _stk.enter_context(tc.tile_pool(name="nati", bufs=2)),
            }
            scrp = _stk.enter_context(tc.tile_pool(name="scr", bufs=2))
            psT = _stk.enter_context(
                tc.tile_pool(name="psT", bufs=2, space="PSUM"))
            psX = _stk.enter_context(
                tc.tile_pool(name="psX", bufs=2, space="PSUM"))
            mean_stk = ExitStack()
            psM = mean_stk.enter_context(
                tc.tile_pool(name="psM", bufs=1, space="PSUM"))
            mean_ps = {"r": psM.tile([1, BC], FP, tag="mean_r",
                                     name="mean_ps_r"),
                       "i": psM.tile([1, BC], FP, tag="mean_i",
                                     name="mean_ps_i")}

            xdr = {"r": x_r, "i": x_i}
            # copy-engine rotation for the PSUM->SBUF transpose copies
            copy_rot = ["v", "v", "v", "a", "v", "v", "v", "a"]
            cov_h = [None, None]
            ag_wait = []

            # ======== Phase A: stream f-slices; transpose, T, S, mean ======
            for s in range(NSP):
                t0 = 2 * s
                nat = {}
                for ch in ("r", "i"):
                    n = natp[ch].tile([128, 8 * 256], FPR, tag="nat")
                    src = xdr[ch][:, 256 * s:256 * (s + 1)].rearrange(
                        "(a p) f -> p a f", p=128)
                    dst = n[:].rearrange("p (a f) -> p a f", f=256)
                    nc.sync.dma_start(dst, src)
                    nat[ch] = n

                # --- T[c,f] via fp32r matmuls, one uninterrupted group per
                # (channel, chunk); the T-quadratic corr terms read the PSUM
                # tiles directly (no SBUF copy of T needed).
                for tt in range(2):
                    t = t0 + tt
                    pT = {}
                    for ch in ("r", "i"):
                        pTc = psT.tile([128, CSH], FP, tag="pT",
                                       name=f"pT_{ch}_{t}")
                        for b in range(8):
                            lhsT = nat[ch][:, 256 * b + 128 * tt:
                                           256 * b + 128 * (tt + 1)]
                            nc.tensor.matmul(pTc[:], lhsT, sel_t[:],
                                             start=(b == 0), stop=(b == 7))
                        pT[ch] = pTc
                    ts1 = scrp.tile([128, CSH], FP, tag="tsq")
                    nc.scalar.activation(ts1[:], pT["r"][:], AF.Square,
                                         accum_out=corr_rr[:, t:t + 1])
                    ts2 = scrp.tile([128, CSH], FP, tag="tsq")
                    nc.scalar.activation(ts2[:], pT["i"][:], AF.Square,
                                         accum_out=corr_ii[:, t:t + 1])
                    tr_sb = scrp.tile([128, CSH], FP, tag="tsq")
                    nc.vector.tensor_copy(tr_sb[:], pT["r"][:])
                    ts3 = scrp.tile([128, CSH], FP, tag="tsq")
                    nc.vector.scalar_tensor_tensor(
                        out=ts3[:], in0=tr_sb[:], scalar=1.0,
                        in1=pT["i"][:], op0=OP.mult, op1=OP.mult,
                        accum_out=corr_ri[:, t:t + 1],
                    )

                # --- transposes: 4 per psum tile (2 b x 2 chunks), then one
                # strided 512-col copy into xT.
                ci = 0
                for ch in ("r", "i"):
                    for bp in range(4):
                        px = psX.tile([128, 512], FPR, tag="px")
                        for bb in range(2):
                            for tt in range(2):
                                b = 2 * bp + bb
                                nc.tensor.transpose(
                                    px[:, 256 * tt + 128 * bb:
                                       256 * tt + 128 * bb + 128],
                                    nat[ch][:, 256 * b + 128 * tt:
                                            256 * b + 128 * (tt + 1)],
                                    ident_t[:],
                                )
                        dst = xT[ch][:].rearrange(
                            "p (t j) -> p t j", j=BC
                        )[:, t0:t0 + 2, 256 * bp:256 * (bp + 1)]
                        srcv = px[:].rearrange("p (t q) -> p t q", q=256)
                        eng = copy_rot[ci]
                        ci += 1
                        if eng == "v":
                            nc.vector.tensor_copy(dst, srcv)
                        elif eng == "a":
                            nc.scalar.copy(dst, srcv)
                        else:
                            nc.gpsimd.tensor_copy(dst, srcv)

                # --- mean over F: negated ones-matmuls accumulated in PSUM
                # across all chunks (groups span phase A; own psum banks).
                for ch in ("r", "i"):
                    for tt in range(2):
                        t = t0 + tt
                        for h in range(2):
                            nc.tensor.matmul(
                                mean_ps[ch][:, 512 * h:512 * (h + 1)],
                                onesF_t[:],
                                xT[ch][:, BC * t + 512 * h:
                                       BC * t + 512 * (h + 1)],
                                start=(t == 0), stop=(t == NFT - 1),
                            )

                # --- second moments per chunk (full-bc accumulation in one
                # instruction each) + T-quadratic correction.
                for tt in range(2):
                    t = t0 + tt
                    sl = slice(BC * t, BC * (t + 1))
                    sc1 = scrp.tile([128, BC], FP, tag="sq")
                    nc.scalar.activation(sc1[:], xT_r[:, sl], AF.Square,
                                         accum_out=S_rr[:, t:t + 1])
                    sc2 = scrp.tile([128, BC], FP, tag="sq")
                    nc.scalar.activation(sc2[:], xT_i[:, sl], AF.Square,
                                         accum_out=S_ii[:, t:t + 1])
                    sc3 = scrp.tile([128, BC], FP, tag="sq")
                    nc.vector.scalar_tensor_tensor(
                        out=sc3[:], in0=xT_r[:, sl], scalar=1.0,
                        in1=xT_i[:, sl], op0=OP.mult, op1=OP.mult,
                        accum_out=S_ri[:, t:t + 1],
                    )

                # --- at half boundaries: pack raw partial cov, AllGather it
                # (bounce DMAs on the gpsimd SWDGE ring so they never block
                # the SP input-load stream).
                if s in (3, 7):
                    h = s // 4
                    hs = slice(8 * h, 8 * (h + 1))
                    partial = small.tile([128, 24], FP, tag=f"partial{h}",
                                         name=f"partial{h}")
                    for m, (S, corr) in enumerate(
                        ((S_rr, corr_rr), (S_ri, corr_ri), (S_ii, corr_ii))
                    ):
                        nc.vector.scalar_tensor_tensor(
                            out=partial[:, 8 * m:8 * (m + 1)],
                            in0=corr[:, hs], scalar=-1.0 / B, in1=S[:, hs],
                            op0=OP.mult, op1=OP.add,
                        )
                    ag_in = dram.tile([128, 24], FP, tag=f"ag_in{h}",
                                      name=f"ag_in{h}")
                    ag_out = dram.tile([8, 128 * 24], FP, tag=f"ag_out{h}",
                                       name=f"ag_out{h}")
                    nc.gpsimd.dma_start(ag_in[:], partial[:])
                    nc.gpsimd.collective_compute(
                        "AllGather", OP.bypass,
                        replica_groups=[list(range(NCORES))],
                        ins=[ag_in[:]],
                        outs=[ag_out[:]],
                    )
                    ag_wait.append(ag_out)

            # ======== M3: copy negated means out of PSUM, DMA into rows ====
            for r, ch in ((0, "r"), (1, "i")):
                row = small.tile([1, BC], BF, tag="rowtmp", name=f"row{ch}")
                nc.vector.tensor_copy(row[:], mean_ps[ch][:])
                nc.sync.dma_start(M3[r:r + 1, :], row[:])
            mean_stk.close()
            _stk.close()

            # beta / ones rows (bf16, on the SP ring which is idle once the
            # input loads drain; they only gate the K3 matmuls of phase D).
            nc.sync.dma_start(A3C[2:3, :], beta_ilv[:])
            nc.sync.dma_start(M3[2:3, :], ones_bc[:])

            # ======== Phases C+D interleaved per half, so half 0 applies
            # while half 1's AllGather is still in flight.
            a_rr = small.tile([128, NFT], FP, tag="a_rr")
            a_ri = small.tile([128, NFT], FP, tag="a_ri")
            a_ir = small.tile([128, NFT], FP, tag="a_ir")
            a_ii = small.tile([128, NFT], FP, tag="a_ii")

            _stk3 = ExitStack()
            pso = _stk3.enter_context(
                tc.tile_pool(name="pso", bufs=5, space="PSUM"))
            stgp = _stk3.enter_context(tc.tile_pool(name="stg", bufs=3))
            copy_flip = 0

            for h in range(2):
                hs = slice(8 * h, 8 * (h + 1))
                # gather result -> SBUF (gpsimd ring; Pool is idle by now)
                # and tree-reduce the 8 replica partials.
                gat = small.tile([128, 8 * 24], FP, tag=f"gat{h}",
                                 name=f"gat{h}")
                nc.gpsimd.dma_start(
                    gat[:].rearrange("p (r j) -> p r j", j=24),
                    ag_wait[h][:].rearrange("r (p j) -> p r j", p=128),
                )
                nc.vector.tensor_tensor(out=gat[:, 0:96], in0=gat[:, 0:96],
                                        in1=gat[:, 96:192], op=OP.add)
                nc.vector.tensor_tensor(out=gat[:, 0:48], in0=gat[:, 0:48],
                                        in1=gat[:, 48:96], op=OP.add)
                cov = small.tile([128, 24], FP, tag=f"cov{h}",
                                 name=f"cov{h}")
                nc.vector.tensor_tensor(out=cov[:], in0=gat[:, 0:24],
                                        in1=gat[:, 24:48], op=OP.add)

                def stile(tag):
                    return small.tile([128, 8], FP, tag=tag,
                                      name=f"{tag}_{h}")

                arr, bri, cii = stile("arr"), stile("bri"), stile("cii")
                nc.vector.tensor_scalar(out=arr[:], in0=cov[:, 0:8],
                                        scalar1=1.0 / NM1, scalar2=EPS,
                                        op0=OP.mult, op1=OP.add)
                nc.vector.tensor_scalar(out=bri[:], in0=cov[:, 8:16],
                                        scalar1=1.0 / NM1, scalar2=None,
                                        op0=OP.mult)
                nc.vector.tensor_scalar(out=cii[:], in0=cov[:, 16:24],
                                        scalar1=1.0 / NM1, scalar2=EPS,
                                        op0=OP.mult, op1=OP.add)

                det, tmp = stile("det"), stile("tmp")
                nc.vector.tensor_tensor(out=det[:], in0=arr[:], in1=cii[:],
                                        op=OP.mult)
                nc.vector.tensor_tensor(out=tmp[:], in0=bri[:], in1=bri[:],
                                        op=OP.mult)
                nc.vector.tensor_tensor(out=det[:], in0=det[:], in1=tmp[:],
                                        op=OP.subtract)
                s_t = stile("s_t")
                nc.scalar.activation(s_t[:], det[:], AF.Sqrt)
                tsum = stile("tsum")
                nc.vector.tensor_tensor(out=tsum[:], in0=arr[:], in1=cii[:],
                                        op=OP.add)
                nc.vector.scalar_tensor_tensor(
                    out=tsum[:], in0=s_t[:], scalar=2.0, in1=tsum[:],
                    op0=OP.mult, op1=OP.add)
                tval = stile("tval")
                nc.scalar.activation(tval[:], tsum[:], AF.Sqrt)
                den, rden = stile("den"), stile("rden")
                nc.vector.tensor_tensor(out=den[:], in0=s_t[:], in1=tval[:],
                                        op=OP.mult)
                nc.vector.reciprocal(rden[:], den[:])

                w_rr, w_ii, wri_n = stile("w_rr"), stile("w_ii"), stile("wri")
                nc.vector.tensor_tensor(out=w_rr[:], in0=cii[:], in1=s_t[:],
                                        op=OP.add)
                nc.vector.tensor_tensor(out=w_rr[:], in0=w_rr[:], in1=rden[:],
                                        op=OP.mult)
                nc.vector.tensor_tensor(out=w_ii[:], in0=arr[:], in1=s_t[:],
                                        op=OP.add)
                nc.vector.tensor_tensor(out=w_ii[:], in0=w_ii[:], in1=rden[:],
                                        op=OP.mult)
                nc.vector.scalar_tensor_tensor(
                    out=wri_n[:], in0=bri[:], scalar=-1.0, in1=rden[:],
                    op0=OP.mult, op1=OP.mult)

                u, v = stile("u"), stile("v")
                for a_t, gx, wx, gy, wy, opc in (
                    (a_rr, g_r_t, w_rr, g_i_t, wri_n, OP.subtract),
                    (a_ri, g_r_t, wri_n, g_i_t, w_ii, OP.subtract),
                    (a_ir, g_i_t, w_rr, g_r_t, wri_n, OP.add),
                    (a_ii, g_i_t, wri_n, g_r_t, w_ii, OP.add),
                ):
                    nc.vector.tensor_tensor(out=u[:], in0=gx[:, hs],
                                            in1=wx[:], op=OP.mult)
                    nc.vector.tensor_tensor(out=v[:], in0=gy[:, hs],
                                            in1=wy[:], op=OP.mult)
                    nc.vector.tensor_tensor(out=a_t[:, hs], in0=u[:],
                                            in1=v[:], op=opc)

                # A3C rows for this half: bf16 staging copy, DRAM bounce,
                # strided re-read into the interleaved row layout.
                for row, (ev, od) in enumerate(((a_rr, a_ir), (a_ri, a_ii))):
                    for cpar, srct in ((0, ev), (1, od)):
                        abf = small.tile([128, 8], BF,
                                         tag=f"abf{h}{row}{cpar}",
                                         name=f"abf{h}{row}{cpar}")
                        nc.vector.tensor_copy(abf[:], srct[:, hs])
                        db = dram.tile([128, 8], BF, tag=f"db{h}{row}{cpar}",
                                       name=f"db{h}{row}{cpar}")
                        nc.sync.dma_start(db[:], abf[:])
                        src = db[:].rearrange("p t -> (p t)").rearrange(
                            "(p t) -> t p", p=128, t=8)
                        dst = A3C[row:row + 1,
                                  2048 * h + cpar:2048 * (h + 1):2].rearrange(
                            "z (t p) -> z t p", t=8, p=128)
                        nc.sync.dma_start(dst, src)

                # ---- Phase D for this half: diag-W apply, stage, store
                for t2 in range(4 * h, 4 * h + 4):
                    ta, tb = 2 * t2, 2 * t2 + 1
                    Ws = []
                    for t in (ta, tb):
                        W_r = wpool.tile([128, 256], FPR, tag="W_r",
                                         name=f"W_r_{t}")
                        W_i = wpool.tile([128, 256], FPR, tag="W_i",
                                         name=f"W_i_{t}")
                        for W, (ev, od) in ((W_r, (a_rr, a_ir)),
                                            (W_i, (a_ri, a_ii))):
                            Wv = W[:].rearrange("p (g c) -> p g c", c=2)
                            nc.vector.tensor_scalar(
                                out=Wv[:, :, 0], in0=ident_t[:],
                                scalar1=ev[:, t:t + 1], scalar2=None,
                                op0=OP.mult,
                            )
                            nc.vector.tensor_scalar(
                                out=Wv[:, :, 1], in0=ident_t[:],
                                scalar1=od[:, t:t + 1], scalar2=None,
                                op0=OP.mult,
                            )
                        Ws.append((W_r, W_i))
                    for bp in range(4):
                        stg = stgp.tile([128, 1024], FP, tag="stg")
                        for bb in range(2):
                            b = 2 * bp + bb
                            po = pso.tile([128, 512], FP, tag="po")
                            nc.tensor.matmul(
                                po[:],
                                M3[:, 128 * b:128 * (b + 1)],
                                A3C[:, 512 * t2:512 * (t2 + 1)],
                                start=True, stop=False,
                            )
                            for j, t in enumerate((ta, tb)):
                                W_r, W_i = Ws[j]
                                sl = slice(BC * t + 128 * b,
                                           BC * t + 128 * (b + 1))
                                nc.tensor.matmul(
                                    po[:, 256 * j:256 * (j + 1)],
                                    xT_r[:, sl], W_r[:],
                                    start=False, stop=False,
                                )
                                nc.tensor.matmul(
                                    po[:, 256 * j:256 * (j + 1)],
                                    xT_i[:, sl], W_i[:],
                                    start=False, stop=(j == 1),
                                )
                            if copy_flip % 2 == 0:
                                nc.vector.tensor_copy(
                                    stg[:, 512 * bb:512 * (bb + 1)], po[:])
                            else:
                                nc.scalar.copy(
                                    stg[:, 512 * bb:512 * (bb + 1)], po[:])
                            copy_flip += 1
                        dst = out.rearrange("(a p) f -> p a f", p=128)[
                            :, 2 * bp:2 * (bp + 1), 512 * t2:512 * (t2 + 1)
                        ]
                        src = stg[:].rearrange("p (a q) -> p a q", q=512)
                        if (4 * t2 + bp) % 2 == 0:
                            nc.scalar.dma_start(dst, src)
                        else:
                            nc.sync.dma_start(dst, src)
            _stk3.close()

    split_multi_waits(nc)
    return nc


_CACHE = {}


def _get_nc():
    if "nc" not in _CACHE:
        _CACHE["nc"] = build_bass()
    return _CACHE["nc"]


def _constants():
    if "consts" not in _CACHE:
        sel = np.zeros((128, CSH), dtype=np.float32)
        for p in range(128):
            sel[p, p % CSH] = 1.0
        _CACHE["consts"] = {
            "ident": np.eye(128, dtype=np.float32),
            "seldr": np.ascontiguousarray(sel),
            "onesF": np.full((128, 1), -1.0 / F, dtype=np.float32),
            "ones_bc": np.ones((1, BC), dtype=np.float32),
        }
    return _CACHE["consts"]


def kernel(x_real, x_imag, gamma_r, gamma_i, beta_r, beta_i):
    x_real = np.ascontiguousarray(x_real, dtype=np.float32)
    x_imag = np.ascontiguousarray(x_imag, dtype=np.float32)
    gamma_r = np.asarray(gamma_r, dtype=np.float32)
    gamma_i = np.asarray(gamma_i, dtype=np.float32)
    beta_r = np.asarray(beta_r, dtype=np.float32)
    beta_i = np.asarray(beta_i, dtype=np.float32)

    nc = _get_nc()
    consts = _constants()
    g_r_t = np.ascontiguousarray(gamma_r.reshape(NFT, 128).T)
    g_i_t = np.ascontiguousarray(gamma_i.reshape(NFT, 128).T)
    import ml_dtypes
    beta_ilv = np.ascontiguousarray(
        np.stack([beta_r, beta_i], axis=-1).reshape(1, 2 * F)
    ).astype(ml_dtypes.bfloat16)

    in_maps = []
    for k in range(NCORES):
        cs = slice(CSH * k, CSH * (k + 1))
        in_maps.append({
            "x_r": np.ascontiguousarray(
                x_real[:, cs, :].reshape(BC, F)),
            "x_i": np.ascontiguousarray(
                x_imag[:, cs, :].reshape(BC, F)),
            "g_r": g_r_t, "g_i": g_i_t, "beta_ilv": beta_ilv,
            **consts,
        })

    res = run_bass_kernel_spmd(nc, in_maps, list(range(NCORES)))

    full = np.empty((B, C, F, 2), dtype=np.float32)
    for k in range(NCORES):
        full[:, CSH * k:CSH * (k + 1)] = (
            res.results[k]["out"].reshape(B, CSH, F, 2)
        )
    return full
